# revision 7
# baseline (speedup 1.0000x reference)
"""MultiHeadAttention Trainium2 kernel (8 NeuronCores, SPMD).

Sharding: core = b*4 + hg where b = batch (0..1), hg = head-group (0..3).
Each core handles 4 heads (256 features) of one batch:
  Q^T/K^T = Wq/Wk column-shard proj (feature-major), V token-major,
  scores^T = K Q^T (softmax denominator via ones-augmented V matmul),
  partial out = ctx @ Wo^T row-shard.  Host sums the 4 partials per batch.

All matmuls bf16 with fp32 PSUM accumulation. exp on ScalarE (scale=1/8
fused), copies + bias adds on VectorE. Biases handled exactly:
  bq/bk: per-partition adds on Q^T/K^T (feature-major layout)
  bv, bo: folded into bo_eff = Wo[:,fsel] @ bv[fsel] + (hg==0)*bo since
          sum_k softmax = 1  =>  (ctx+bv) @ WoT = ctx @ WoT + bv @ WoT.
"""

import sys

if "/opt/trn_rl_repo" not in sys.path:
    sys.path.insert(0, "/opt/trn_rl_repo")

from contextlib import ExitStack

import ml_dtypes
import numpy as np

import concourse.bass as bass
import concourse.tile as tile
from concourse import bacc, mybir

BF16 = mybir.dt.bfloat16
F32 = mybir.dt.float32
NP_BF16 = ml_dtypes.bfloat16

B, S, D = 2, 2048, 1024
H, HD = 16, 64
N_CORES = 8
HPC = 4          # heads per core
FPC = HPC * HD   # features per core = 256
P = 128
SC = 512         # q-chunk for scores/ctx matmuls (one PSUM bank)
QC = S // SC     # 4 q-chunks
DT = D // P      # 8 d-tiles (contraction tiles for projections)
KT = S // P      # 16 k-token tiles
ST = S // P      # 16 s-token tiles (V)
FT = FPC // P    # 2 feature tiles per core (Q^T/K^T, ctx)
OT = D // P      # 8 output feature tiles


def build_program():
    nc = bacc.Bacc("TRN2", target_bir_lowering=False, debug=False,
                   num_devices=N_CORES)

    qT = nc.declare_dram_parameter("qT", [D, S], BF16, isOutput=False)
    kT = nc.declare_dram_parameter("kT", [D, S], BF16, isOutput=False)
    vT = nc.declare_dram_parameter("vT", [D, S], BF16, isOutput=False)
    wqT = nc.declare_dram_parameter("wqT", [D, FPC], BF16, isOutput=False)
    wkT = nc.declare_dram_parameter("wkT", [D, FPC], BF16, isOutput=False)
    wvT = nc.declare_dram_parameter("wvT", [D, FPC], BF16, isOutput=False)
    woT = nc.declare_dram_parameter("woT", [FPC, D], BF16, isOutput=False)
    bq = nc.declare_dram_parameter("bq", [FPC], F32, isOutput=False)
    bk = nc.declare_dram_parameter("bk", [FPC], F32, isOutput=False)
    bo_eff = nc.declare_dram_parameter("bo_eff", [D], F32, isOutput=False)
    outT = nc.declare_dram_parameter("outT", [D, S], F32, isOutput=True)

    with tile.TileContext(nc) as tc, ExitStack() as ctx:
        persist = ctx.enter_context(tc.tile_pool(name="persist", bufs=1))
        psum_a = ctx.enter_context(
            tc.tile_pool(name="psum_a", bufs=2, space="PSUM"))
        psum_sc = ctx.enter_context(
            tc.tile_pool(name="psum_sc", bufs=2, space="PSUM"))
        psum_ctx = ctx.enter_context(
            tc.tile_pool(name="psum_ctx", bufs=2, space="PSUM"))
        exp_pool = ctx.enter_context(tc.tile_pool(name="exp", bufs=4))
        z_pool = ctx.enter_context(tc.tile_pool(name="z", bufs=4))
        zdram_pool = ctx.enter_context(
            tc.tile_pool(name="zdram", bufs=16, space="DRAM"))
        out_pool = ctx.enter_context(tc.tile_pool(name="out", bufs=4))

        # ---- load inputs / weights ----------------------------------
        qT_sb = [persist.tile([P, S], BF16, tag=f"qT{i}", name=f"qT{i}") for i in range(DT)]
        kT_sb = [persist.tile([P, S], BF16, tag=f"kT{i}", name=f"kT{i}") for i in range(DT)]
        vT_sb = [persist.tile([P, S], BF16, tag=f"vT{i}", name=f"vT{i}") for i in range(DT)]
        wq_sb = [persist.tile([P, FPC], BF16, tag=f"wq{i}", name=f"wq{i}") for i in range(DT)]
        wk_sb = [persist.tile([P, FPC], BF16, tag=f"wk{i}", name=f"wk{i}") for i in range(DT)]
        wv_sb = [persist.tile([P, FPC], BF16, tag=f"wv{i}", name=f"wv{i}") for i in range(DT)]
        wo_sb = [persist.tile([P, D], BF16, tag=f"wo{i}", name=f"wo{i}") for i in range(FT)]
        for i in range(DT):
            nc.sync.dma_start(out=qT_sb[i], in_=qT[i * P:(i + 1) * P, :])
            nc.sync.dma_start(out=kT_sb[i], in_=kT[i * P:(i + 1) * P, :])
            nc.sync.dma_start(out=vT_sb[i], in_=vT[i * P:(i + 1) * P, :])
            nc.sync.dma_start(out=wq_sb[i], in_=wqT[i * P:(i + 1) * P, :])
            nc.sync.dma_start(out=wk_sb[i], in_=wkT[i * P:(i + 1) * P, :])
            nc.sync.dma_start(out=wv_sb[i], in_=wvT[i * P:(i + 1) * P, :])
        for i in range(FT):
            nc.sync.dma_start(out=wo_sb[i], in_=woT[i * P:(i + 1) * P, :])

        bq_sb = persist.tile([P, FT], F32, tag="bq")
        bk_sb = persist.tile([P, FT], F32, tag="bk")
        bo_sb = persist.tile([P, OT], F32, tag="bo")
        nc.sync.dma_start(out=bq_sb, in_=bq.rearrange("(t p) -> p t", p=P))
        nc.sync.dma_start(out=bk_sb, in_=bk.rearrange("(t p) -> p t", p=P))
        nc.sync.dma_start(out=bo_sb, in_=bo_eff.rearrange("(t p) -> p t", p=P))

        # ---- phase A: projections -----------------------------------
        # Q^T, K^T feature-major [FPC, S]
        QT_sb = [persist.tile([P, S], BF16, tag=f"QT{t}", name=f"QT{t}") for t in range(FT)]
        KT_sb = [persist.tile([P, S], BF16, tag=f"KT{t}", name=f"KT{t}") for t in range(FT)]
        for name, w_sb, x_sb, b_sb, dst in (
            ("q", wq_sb, qT_sb, bq_sb, QT_sb),
            ("k", wk_sb, kT_sb, bk_sb, KT_sb),
        ):
            for t in range(FT):
                for c in range(QC):
                    ps = psum_a.tile([P, SC], F32, tag="mm512")
                    for d in range(DT):
                        nc.tensor.matmul(
                            out=ps,
                            lhsT=w_sb[d][:, t * P:(t + 1) * P],
                            rhs=x_sb[d][:, c * SC:(c + 1) * SC],
                            start=(d == 0), stop=(d == DT - 1))
                    nc.vector.tensor_scalar_add(
                        out=dst[t][:, c * SC:(c + 1) * SC],
                        in0=ps, scalar1=b_sb[:, t:t + 1])

        # V token-major, ones-augmented: [S, 4*(HD+1)]; col 64 of each
        # head group is 1.0 so the ctx matmul also produces the softmax
        # denominator Z as output row HD.
        V_sb = [persist.tile([P, HPC * (HD + 1)], BF16, tag=f"V{i}", name=f"V{i}")
                for i in range(ST)]
        for st in range(ST):
            ps = psum_a.tile([P, FPC], F32, tag="mm512")
            for d in range(DT):
                nc.tensor.matmul(
                    out=ps,
                    lhsT=vT_sb[d][:, st * P:(st + 1) * P],
                    rhs=wv_sb[d],
                    start=(d == 0), stop=(d == DT - 1))
            vv = V_sb[st].rearrange("p (h x) -> p h x", h=HPC)
            nc.vector.tensor_copy(
                out=vv[:, :, 0:HD],
                in_=ps.rearrange("p (h x) -> p h x", x=HD))
            nc.vector.memset(vv[:, :, HD:HD + 1], 1.0)

        # ---- phase B: attention (2 head-pairs x 4 q-chunks) ---------
        # ctx^T feature-major [FPC, S] bf16 (normalized)
        ctxn_sb = [persist.tile([P, S], BF16, tag=f"ctxn{t}", name=f"ctxn{t}")
                   for t in range(FT)]
        for hp in range(FT):          # head pair = Q^T/K^T tile index
            for c in range(QC):
                cps = [psum_ctx.tile([HD + 1, SC], F32, tag="ctx", name="cps")
                       for _ in range(2)]
                for kt in range(KT):
                    sc = psum_sc.tile([P, 2 * SC], F32, tag="sc")
                    ex = exp_pool.tile([P, 2 * SC], BF16, tag="ex")
                    for i in range(2):  # head 2*hp + i at partitions 64*i
                        hp0 = HD * i
                        nc.tensor.matmul(
                            out=sc[:, i * SC:(i + 1) * SC],
                            lhsT=KT_sb[hp][hp0:hp0 + HD,
                                           kt * P:(kt + 1) * P],
                            rhs=QT_sb[hp][hp0:hp0 + HD,
                                          c * SC:(c + 1) * SC],
                            start=True, stop=True)
                    nc.scalar.activation(
                        out=ex, in_=sc,
                        func=mybir.ActivationFunctionType.Exp,
                        scale=1.0 / (HD ** 0.5))
                    for i in range(2):
                        h = 2 * hp + i
                        nc.tensor.matmul(
                            out=cps[i],
                            lhsT=V_sb[kt][:, h * (HD + 1):(h + 1) * (HD + 1)],
                            rhs=ex[:, i * SC:(i + 1) * SC],
                            start=(kt == 0), stop=(kt == KT - 1))
                # normalize: ctx_h / Z_h (+ move odd head to partitions 64+)
                for i in range(2):
                    zr = z_pool.tile([HD + 1, SC], F32, tag="zr", name="zr")
                    nc.vector.reciprocal(
                        out=zr[HD:HD + 1, :], in_=cps[i][HD:HD + 1, :])
                    zd = zdram_pool.tile([1, SC], F32, tag="zd", name="zd")
                    nc.sync.dma_start(out=zd, in_=zr[HD:HD + 1, :])
                    rz = z_pool.tile([HD, SC], F32, tag="rz", name="rz")
                    nc.sync.dma_start(
                        out=rz, in_=zd.partition_broadcast(HD))
                    if i == 0:
                        nc.vector.tensor_mul(
                            out=ctxn_sb[hp][0:HD, c * SC:(c + 1) * SC],
                            in0=cps[i][0:HD, :], in1=rz)
                    else:
                        tmp = z_pool.tile([HD, SC], BF16, tag="tmp")
                        nc.vector.tensor_mul(
                            out=tmp, in0=cps[i][0:HD, :], in1=rz)
                        nc.sync.dma_start(
                            out=ctxn_sb[hp][HD:P, c * SC:(c + 1) * SC],
                            in_=tmp)

        # ---- phase C: output projection (partial) -------------------
        for fo in range(OT):
            for c in range(QC):
                ps = psum_a.tile([P, SC], F32, tag="mm512")
                for t in range(FT):
                    nc.tensor.matmul(
                        out=ps,
                        lhsT=wo_sb[t][:, fo * P:(fo + 1) * P],
                        rhs=ctxn_sb[t][:, c * SC:(c + 1) * SC],
                        start=(t == 0), stop=(t == FT - 1))
                ob = out_pool.tile([P, SC], F32, tag="ob")
                nc.vector.tensor_scalar_add(
                    out=ob, in0=ps, scalar1=bo_sb[:, fo:fo + 1])
                nc.sync.dma_start(
                    out=outT[fo * P:(fo + 1) * P, c * SC:(c + 1) * SC],
                    in_=ob)

    nc.compile()
    return nc


_NC_CACHE = None


def _get_program():
    global _NC_CACHE
    if _NC_CACHE is None:
        _NC_CACHE = build_program()
    return _NC_CACHE


def make_in_maps(query, key, value, Wq, bq, Wk, bk, Wv, bv, Wo, bo):
    query = np.asarray(query, np.float32)
    key = np.asarray(key, np.float32)
    value = np.asarray(value, np.float32)
    Wq, bq = np.asarray(Wq, np.float32), np.asarray(bq, np.float32)
    Wk, bk = np.asarray(Wk, np.float32), np.asarray(bk, np.float32)
    Wv, bv = np.asarray(Wv, np.float32), np.asarray(bv, np.float32)
    Wo, bo = np.asarray(Wo, np.float32), np.asarray(bo, np.float32)

    in_maps = []
    for core in range(N_CORES):
        b, hg = core // 4, core % 4
        fs = slice(FPC * hg, FPC * (hg + 1))
        in_maps.append({
            "qT": np.ascontiguousarray(query[b].T).astype(NP_BF16),
            "kT": np.ascontiguousarray(key[b].T).astype(NP_BF16),
            "vT": np.ascontiguousarray(value[b].T).astype(NP_BF16),
            "wqT": np.ascontiguousarray(Wq[fs, :].T).astype(NP_BF16),
            "wkT": np.ascontiguousarray(Wk[fs, :].T).astype(NP_BF16),
            "wvT": np.ascontiguousarray(Wv[fs, :].T).astype(NP_BF16),
            "woT": np.ascontiguousarray(Wo[:, fs].T).astype(NP_BF16),
            "bq": np.ascontiguousarray(bq[fs]),
            "bk": np.ascontiguousarray(bk[fs]),
            "bo_eff": np.ascontiguousarray(
                Wo[:, fs] @ bv[fs] + (bo if hg == 0 else 0.0)),
        })
    return in_maps


def gather_output(results):
    out = np.zeros((B, S, D), np.float32)
    for core in range(N_CORES):
        out[core // 4] += results[core]["outT"].T
    return out


def kernel(**inputs):
    from concourse.bass_utils import run_bass_kernel_spmd

    nc = _get_program()
    in_maps = make_in_maps(**inputs)
    res = run_bass_kernel_spmd(nc, in_maps, list(range(N_CORES)))
    return gather_output(res.results)


if __name__ == "__main__":
    import jax

    sys.path.insert(0, "/root/problem")
    import reference

    inputs = {k: np.asarray(v) for k, v in reference.setup_inputs().items()}
    expected = np.asarray(reference.reference(**inputs))
    actual = kernel(**inputs)
    err = np.linalg.norm(actual - expected) / np.linalg.norm(expected)
    print("Relative error:", err)


# revision 10
# speedup vs baseline: 1.0411x; 1.0411x over previous
"""MultiHeadAttention Trainium2 kernel (8 NeuronCores, SPMD).

Sharding: core = b*4 + hg where b = batch (0..1), hg = head-group (0..3).
Each core handles 4 heads (256 features) of one batch:
  Q^T/K^T = Wq/Wk column-shard proj (feature-major), V token-major,
  scores^T = K Q^T (softmax denominator via ones-augmented V matmul),
  partial out = ctx @ Wo^T row-shard.  Host sums the 4 partials per batch.

All matmuls bf16 with fp32 PSUM accumulation. exp on ScalarE (scale=1/8
fused), copies + bias adds on VectorE. Biases handled exactly:
  bq/bk: per-partition adds on Q^T/K^T (feature-major layout)
  bv, bo: folded into bo_eff = Wo[:,fsel] @ bv[fsel] + (hg==0)*bo since
          sum_k softmax = 1  =>  (ctx+bv) @ WoT = ctx @ WoT + bv @ WoT.
"""

import sys

if "/opt/trn_rl_repo" not in sys.path:
    sys.path.insert(0, "/opt/trn_rl_repo")

from contextlib import ExitStack

import ml_dtypes
import numpy as np

import concourse.bass as bass
import concourse.tile as tile
from concourse import bacc, mybir

BF16 = mybir.dt.bfloat16
F32 = mybir.dt.float32
NP_BF16 = ml_dtypes.bfloat16

B, S, D = 2, 2048, 1024
H, HD = 16, 64
N_CORES = 8
HPC = 4          # heads per core
FPC = HPC * HD   # features per core = 256
P = 128
SC = 512         # q-chunk for scores/ctx matmuls (one PSUM bank)
QC = S // SC     # 4 q-chunks
DT = D // P      # 8 d-tiles (contraction tiles for projections)
KT = S // P      # 16 k-token tiles
ST = S // P      # 16 s-token tiles (V)
FT = FPC // P    # 2 feature tiles per core (Q^T/K^T, ctx)
OT = D // P      # 8 output feature tiles


def build_program():
    nc = bacc.Bacc("TRN2", target_bir_lowering=False, debug=False,
                   num_devices=N_CORES)

    qT = nc.declare_dram_parameter("qT", [D, S], BF16, isOutput=False)
    kT = nc.declare_dram_parameter("kT", [D, S], BF16, isOutput=False)
    vT = nc.declare_dram_parameter("vT", [D, S], BF16, isOutput=False)
    wqT = nc.declare_dram_parameter("wqT", [D, FPC], BF16, isOutput=False)
    wkT = nc.declare_dram_parameter("wkT", [D, FPC], BF16, isOutput=False)
    wvT = nc.declare_dram_parameter("wvT", [D, FPC], BF16, isOutput=False)
    woT = nc.declare_dram_parameter("woT", [FPC, D], BF16, isOutput=False)
    bq = nc.declare_dram_parameter("bq", [FPC], F32, isOutput=False)
    bk = nc.declare_dram_parameter("bk", [FPC], F32, isOutput=False)
    bo_eff = nc.declare_dram_parameter("bo_eff", [D], F32, isOutput=False)
    outT = nc.declare_dram_parameter("outT", [D, S], F32, isOutput=True)

    with tile.TileContext(nc) as tc, ExitStack() as ctx:
        persist = ctx.enter_context(tc.tile_pool(name="persist", bufs=1))
        psum_a = ctx.enter_context(
            tc.tile_pool(name="psum_a", bufs=2, space="PSUM"))
        psum_sc = ctx.enter_context(
            tc.tile_pool(name="psum_sc", bufs=2, space="PSUM"))
        psum_ctx = ctx.enter_context(
            tc.tile_pool(name="psum_ctx", bufs=2, space="PSUM"))
        exp_pool = ctx.enter_context(tc.tile_pool(name="exp", bufs=4))
        z_pool = ctx.enter_context(tc.tile_pool(name="z", bufs=4))
        zdram_pool = ctx.enter_context(
            tc.tile_pool(name="zdram", bufs=16, space="DRAM"))
        out_pool = ctx.enter_context(tc.tile_pool(name="out", bufs=4))

        # ---- load inputs / weights ----------------------------------
        qT_sb = [persist.tile([P, S], BF16, tag=f"qT{i}", name=f"qT{i}") for i in range(DT)]
        kT_sb = [persist.tile([P, S], BF16, tag=f"kT{i}", name=f"kT{i}") for i in range(DT)]
        vT_sb = [persist.tile([P, S], BF16, tag=f"vT{i}", name=f"vT{i}") for i in range(DT)]
        wq_sb = [persist.tile([P, FPC], BF16, tag=f"wq{i}", name=f"wq{i}") for i in range(DT)]
        wk_sb = [persist.tile([P, FPC], BF16, tag=f"wk{i}", name=f"wk{i}") for i in range(DT)]
        wv_sb = [persist.tile([P, FPC], BF16, tag=f"wv{i}", name=f"wv{i}") for i in range(DT)]
        wo_sb = [persist.tile([P, D], BF16, tag=f"wo{i}", name=f"wo{i}") for i in range(FT)]
        # load order: K first (attention consumes all of K^T), then Q,
        # then V (first needed by the first ctx matmul), then Wo.
        for i in range(DT):
            nc.sync.dma_start(out=wk_sb[i], in_=wkT[i * P:(i + 1) * P, :])
            nc.sync.dma_start(out=kT_sb[i], in_=kT[i * P:(i + 1) * P, :])
        for i in range(DT):
            nc.sync.dma_start(out=wq_sb[i], in_=wqT[i * P:(i + 1) * P, :])
            nc.sync.dma_start(out=qT_sb[i], in_=qT[i * P:(i + 1) * P, :])
        for i in range(DT):
            nc.sync.dma_start(out=wv_sb[i], in_=wvT[i * P:(i + 1) * P, :])
            nc.sync.dma_start(out=vT_sb[i], in_=vT[i * P:(i + 1) * P, :])
        for i in range(FT):
            nc.sync.dma_start(out=wo_sb[i], in_=woT[i * P:(i + 1) * P, :])

        bq_sb = persist.tile([P, FT], F32, tag="bq")
        bk_sb = persist.tile([P, FT], F32, tag="bk")
        bo_sb = persist.tile([P, OT], F32, tag="bo")
        nc.sync.dma_start(out=bq_sb, in_=bq.rearrange("(t p) -> p t", p=P))
        nc.sync.dma_start(out=bk_sb, in_=bk.rearrange("(t p) -> p t", p=P))
        nc.sync.dma_start(out=bo_sb, in_=bo_eff.rearrange("(t p) -> p t", p=P))

        # ---- phase A: projections -----------------------------------
        # Q^T, K^T feature-major [FPC, S]
        QT_sb = [persist.tile([P, S], BF16, tag=f"QT{t}", name=f"QT{t}") for t in range(FT)]
        KT_sb = [persist.tile([P, S], BF16, tag=f"KT{t}", name=f"KT{t}") for t in range(FT)]
        for name, w_sb, x_sb, b_sb, dst in (
            ("k", wk_sb, kT_sb, bk_sb, KT_sb),
            ("q", wq_sb, qT_sb, bq_sb, QT_sb),
        ):
            for t in range(FT):
                for c in range(QC):
                    ps = psum_a.tile([P, SC], F32, tag="mm512")
                    for d in range(DT):
                        nc.tensor.matmul(
                            out=ps,
                            lhsT=w_sb[d][:, t * P:(t + 1) * P],
                            rhs=x_sb[d][:, c * SC:(c + 1) * SC],
                            start=(d == 0), stop=(d == DT - 1))
                    nc.vector.tensor_scalar_add(
                        out=dst[t][:, c * SC:(c + 1) * SC],
                        in0=ps, scalar1=b_sb[:, t:t + 1])

        # V token-major, ones-augmented: [S, 4*(HD+1)]; col 64 of each
        # head group is 1.0 so the ctx matmul also produces the softmax
        # denominator Z as output row HD.
        V_sb = [persist.tile([P, HPC * (HD + 1)], BF16, tag=f"V{i}", name=f"V{i}")
                for i in range(ST)]
        for st in range(ST):
            ps = psum_a.tile([P, FPC], F32, tag="mm512")
            for d in range(DT):
                nc.tensor.matmul(
                    out=ps,
                    lhsT=vT_sb[d][:, st * P:(st + 1) * P],
                    rhs=wv_sb[d],
                    start=(d == 0), stop=(d == DT - 1))
            vv = V_sb[st].rearrange("p (h x) -> p h x", h=HPC)
            nc.vector.tensor_copy(
                out=vv[:, :, 0:HD],
                in_=ps.rearrange("p (h x) -> p h x", x=HD))
            nc.vector.memset(vv[:, :, HD:HD + 1], 1.0)

        # ---- phases B+C: attention + out-proj, interleaved per qc ---
        # ctx^T feature-major [FPC, S] bf16 (normalized)
        ctxn_sb = [persist.tile([P, S], BF16, tag=f"ctxn{t}", name=f"ctxn{t}")
                   for t in range(FT)]
        for c in range(QC):
            for hp in range(FT):      # head pair = Q^T/K^T tile index
                cps = [psum_ctx.tile([HD + 1, SC], F32, tag="ctx", name="cps")
                       for _ in range(2)]
                for kt in range(KT):
                    sc = psum_sc.tile([P, 2 * SC], F32, tag="sc")
                    ex = exp_pool.tile([P, 2 * SC], BF16, tag="ex")
                    for i in range(2):  # head 2*hp + i at partitions 64*i
                        hp0 = HD * i
                        nc.tensor.matmul(
                            out=sc[:, i * SC:(i + 1) * SC],
                            lhsT=KT_sb[hp][hp0:hp0 + HD,
                                           kt * P:(kt + 1) * P],
                            rhs=QT_sb[hp][hp0:hp0 + HD,
                                          c * SC:(c + 1) * SC],
                            start=True, stop=True,
                            tile_position=(hp0, 0))
                    nc.scalar.activation(
                        out=ex, in_=sc,
                        func=mybir.ActivationFunctionType.Exp,
                        scale=1.0 / (HD ** 0.5))
                    for i in range(2):
                        h = 2 * hp + i
                        nc.tensor.matmul(
                            out=cps[i],
                            lhsT=V_sb[kt][:, h * (HD + 1):(h + 1) * (HD + 1)],
                            rhs=ex[:, i * SC:(i + 1) * SC],
                            start=(kt == 0), stop=(kt == KT - 1))
                # normalize: ctx_h / Z_h (+ move odd head to partitions 64+)
                for i in range(2):
                    zr = z_pool.tile([HD + 1, SC], F32, tag="zr", name="zr")
                    nc.vector.reciprocal(
                        out=zr[HD:HD + 1, :], in_=cps[i][HD:HD + 1, :])
                    zd = zdram_pool.tile([1, SC], F32, tag="zd", name="zd")
                    nc.sync.dma_start(out=zd, in_=zr[HD:HD + 1, :])
                    rz = z_pool.tile([HD, SC], F32, tag="rz", name="rz")
                    nc.sync.dma_start(
                        out=rz, in_=zd.partition_broadcast(HD))
                    if i == 0:
                        nc.vector.tensor_mul(
                            out=ctxn_sb[hp][0:HD, c * SC:(c + 1) * SC],
                            in0=cps[i][0:HD, :], in1=rz)
                    else:
                        tmp = z_pool.tile([HD, SC], BF16, tag="tmp")
                        nc.vector.tensor_mul(
                            out=tmp, in0=cps[i][0:HD, :], in1=rz)
                        nc.sync.dma_start(
                            out=ctxn_sb[hp][HD:P, c * SC:(c + 1) * SC],
                            in_=tmp)
            # out-proj for this q-chunk (both head pairs now normalized)
            for fo in range(OT):
                ps = psum_a.tile([P, SC], F32, tag="mm512")
                for t in range(FT):
                    nc.tensor.matmul(
                        out=ps,
                        lhsT=wo_sb[t][:, fo * P:(fo + 1) * P],
                        rhs=ctxn_sb[t][:, c * SC:(c + 1) * SC],
                        start=(t == 0), stop=(t == FT - 1))
                ob = out_pool.tile([P, SC], F32, tag="ob")
                nc.vector.tensor_scalar_add(
                    out=ob, in0=ps, scalar1=bo_sb[:, fo:fo + 1])
                nc.sync.dma_start(
                    out=outT[fo * P:(fo + 1) * P, c * SC:(c + 1) * SC],
                    in_=ob)

    nc.compile()
    return nc


_NC_CACHE = None


def _get_program():
    global _NC_CACHE
    if _NC_CACHE is None:
        _NC_CACHE = build_program()
    return _NC_CACHE


def make_in_maps(query, key, value, Wq, bq, Wk, bk, Wv, bv, Wo, bo):
    query = np.asarray(query, np.float32)
    key = np.asarray(key, np.float32)
    value = np.asarray(value, np.float32)
    Wq, bq = np.asarray(Wq, np.float32), np.asarray(bq, np.float32)
    Wk, bk = np.asarray(Wk, np.float32), np.asarray(bk, np.float32)
    Wv, bv = np.asarray(Wv, np.float32), np.asarray(bv, np.float32)
    Wo, bo = np.asarray(Wo, np.float32), np.asarray(bo, np.float32)

    in_maps = []
    for core in range(N_CORES):
        b, hg = core // 4, core % 4
        fs = slice(FPC * hg, FPC * (hg + 1))
        in_maps.append({
            "qT": np.ascontiguousarray(query[b].T).astype(NP_BF16),
            "kT": np.ascontiguousarray(key[b].T).astype(NP_BF16),
            "vT": np.ascontiguousarray(value[b].T).astype(NP_BF16),
            "wqT": np.ascontiguousarray(Wq[fs, :].T).astype(NP_BF16),
            "wkT": np.ascontiguousarray(Wk[fs, :].T).astype(NP_BF16),
            "wvT": np.ascontiguousarray(Wv[fs, :].T).astype(NP_BF16),
            "woT": np.ascontiguousarray(Wo[:, fs].T).astype(NP_BF16),
            "bq": np.ascontiguousarray(bq[fs]),
            "bk": np.ascontiguousarray(bk[fs]),
            "bo_eff": np.ascontiguousarray(
                Wo[:, fs] @ bv[fs] + (bo if hg == 0 else 0.0)),
        })
    return in_maps


def gather_output(results):
    out = np.zeros((B, S, D), np.float32)
    for core in range(N_CORES):
        out[core // 4] += results[core]["outT"].T
    return out


def kernel(**inputs):
    from concourse.bass_utils import run_bass_kernel_spmd

    nc = _get_program()
    in_maps = make_in_maps(**inputs)
    res = run_bass_kernel_spmd(nc, in_maps, list(range(N_CORES)))
    return gather_output(res.results)


if __name__ == "__main__":
    import jax

    sys.path.insert(0, "/root/problem")
    import reference

    inputs = {k: np.asarray(v) for k, v in reference.setup_inputs().items()}
    expected = np.asarray(reference.reference(**inputs))
    actual = kernel(**inputs)
    err = np.linalg.norm(actual - expected) / np.linalg.norm(expected)
    print("Relative error:", err)


# revision 13
# speedup vs baseline: 1.0858x; 1.0429x over previous
"""MultiHeadAttention Trainium2 kernel (8 NeuronCores, SPMD).

Sharding: core = b*4 + hg where b = batch (0..1), hg = head-group (0..3).
Each core handles 4 heads (256 features) of one batch:
  Q^T/K^T = Wq/Wk column-shard proj (feature-major), V token-major,
  scores^T = K Q^T (softmax denominator via ones-augmented V matmul),
  partial out = ctx @ Wo^T row-shard.  Host sums the 4 partials per batch.

All matmuls bf16 with fp32 PSUM accumulation. exp on ScalarE (scale=1/8
fused), copies + bias adds on VectorE. Biases handled exactly:
  bq/bk: per-partition adds on Q^T/K^T (feature-major layout)
  bv, bo: folded into bo_eff = Wo[:,fsel] @ bv[fsel] + (hg==0)*bo since
          sum_k softmax = 1  =>  (ctx+bv) @ WoT = ctx @ WoT + bv @ WoT.
"""

import sys

if "/opt/trn_rl_repo" not in sys.path:
    sys.path.insert(0, "/opt/trn_rl_repo")

from contextlib import ExitStack

import ml_dtypes
import numpy as np

import concourse.bass as bass
import concourse.tile as tile
from concourse import bacc, mybir

BF16 = mybir.dt.bfloat16
F32 = mybir.dt.float32
NP_BF16 = ml_dtypes.bfloat16

B, S, D = 2, 2048, 1024
H, HD = 16, 64
N_CORES = 8
HPC = 4          # heads per core
FPC = HPC * HD   # features per core = 256
P = 128
SC = 512         # q-chunk for scores/ctx matmuls (one PSUM bank)
QC = S // SC     # 4 q-chunks
DT = D // P      # 8 d-tiles (contraction tiles for projections)
KT = S // P      # 16 k-token tiles
ST = S // P      # 16 s-token tiles (V)
FT = FPC // P    # 2 feature tiles per core (Q^T/K^T, ctx)
OT = D // P      # 8 output feature tiles


def build_program():
    nc = bacc.Bacc("TRN2", target_bir_lowering=False, debug=False,
                   num_devices=N_CORES)

    qT = nc.declare_dram_parameter("qT", [D, S], BF16, isOutput=False)
    kT = nc.declare_dram_parameter("kT", [D, S], BF16, isOutput=False)
    vT = nc.declare_dram_parameter("vT", [D, S], BF16, isOutput=False)
    wqT = nc.declare_dram_parameter("wqT", [D, FPC], BF16, isOutput=False)
    wkT = nc.declare_dram_parameter("wkT", [D, FPC], BF16, isOutput=False)
    wvT = nc.declare_dram_parameter("wvT", [D, FPC], BF16, isOutput=False)
    woT = nc.declare_dram_parameter("woT", [FPC, D], BF16, isOutput=False)
    bq = nc.declare_dram_parameter("bq", [FPC], F32, isOutput=False)
    bk = nc.declare_dram_parameter("bk", [FPC], F32, isOutput=False)
    bo_eff = nc.declare_dram_parameter("bo_eff", [D], F32, isOutput=False)
    outT = nc.declare_dram_parameter("outT", [D, S], F32, isOutput=True)

    with tile.TileContext(nc) as tc, ExitStack() as ctx:
        persist = ctx.enter_context(tc.tile_pool(name="persist", bufs=1))
        psum_a = ctx.enter_context(
            tc.tile_pool(name="psum_a", bufs=2, space="PSUM"))
        psum_sc = ctx.enter_context(
            tc.tile_pool(name="psum_sc", bufs=2, space="PSUM"))
        psum_ctx = ctx.enter_context(
            tc.tile_pool(name="psum_ctx", bufs=2, space="PSUM"))
        exp_pool = ctx.enter_context(tc.tile_pool(name="exp", bufs=4))
        z_pool = ctx.enter_context(tc.tile_pool(name="z", bufs=4))
        zdram_pool = ctx.enter_context(
            tc.tile_pool(name="zdram", bufs=16, space="DRAM"))
        out_pool = ctx.enter_context(tc.tile_pool(name="out", bufs=4))

        # ---- load inputs / weights ----------------------------------
        qT_sb = [persist.tile([P, S], BF16, tag=f"qT{i}", name=f"qT{i}") for i in range(DT)]
        kT_sb = [persist.tile([P, S], BF16, tag=f"kT{i}", name=f"kT{i}") for i in range(DT)]
        vT_sb = [persist.tile([P, S], BF16, tag=f"vT{i}", name=f"vT{i}") for i in range(DT)]
        wq_sb = [persist.tile([P, FPC], BF16, tag=f"wq{i}", name=f"wq{i}") for i in range(DT)]
        wk_sb = [persist.tile([P, FPC], BF16, tag=f"wk{i}", name=f"wk{i}") for i in range(DT)]
        wv_sb = [persist.tile([P, FPC], BF16, tag=f"wv{i}", name=f"wv{i}") for i in range(DT)]
        wo_sb = [persist.tile([P, D], BF16, tag=f"wo{i}", name=f"wo{i}") for i in range(FT)]
        # load order: weights first (small), then K/Q/V activations in
        # 512-column chunks ordered to match projection consumption
        # (first PSUM accumulation needs all 8 d-tiles of one column
        # chunk) so the first matmuls can start after ~1MB, not 4MB.
        for i in range(DT):
            nc.sync.dma_start(out=wk_sb[i], in_=wkT[i * P:(i + 1) * P, :])
            nc.sync.dma_start(out=wq_sb[i], in_=wqT[i * P:(i + 1) * P, :])
            nc.sync.dma_start(out=wv_sb[i], in_=wvT[i * P:(i + 1) * P, :])
        for i in range(FT):
            nc.sync.dma_start(out=wo_sb[i], in_=woT[i * P:(i + 1) * P, :])
        for i in range(DT):
            nc.sync.dma_start(out=kT_sb[i], in_=kT[i * P:(i + 1) * P, :])
        for i in range(DT):
            nc.sync.dma_start(out=qT_sb[i], in_=qT[i * P:(i + 1) * P, :])
        for i in range(DT):
            nc.sync.dma_start(out=vT_sb[i], in_=vT[i * P:(i + 1) * P, :])

        bq_sb = persist.tile([P, FT], F32, tag="bq")
        bk_sb = persist.tile([P, FT], F32, tag="bk")
        bo_sb = persist.tile([P, OT], F32, tag="bo")
        nc.sync.dma_start(out=bq_sb, in_=bq.rearrange("(t p) -> p t", p=P))
        nc.sync.dma_start(out=bk_sb, in_=bk.rearrange("(t p) -> p t", p=P))
        nc.sync.dma_start(out=bo_sb, in_=bo_eff.rearrange("(t p) -> p t", p=P))

        # ---- phase A: projections -----------------------------------
        # Q^T, K^T feature-major [FPC, S]
        QT_sb = [persist.tile([P, S], BF16, tag=f"QT{t}", name=f"QT{t}") for t in range(FT)]
        KT_sb = [persist.tile([P, S], BF16, tag=f"KT{t}", name=f"KT{t}") for t in range(FT)]
        for name, w_sb, x_sb, b_sb, dst in (
            ("k", wk_sb, kT_sb, bk_sb, KT_sb),
            ("q", wq_sb, qT_sb, bq_sb, QT_sb),
        ):
            for t in range(FT):
                for c in range(QC):
                    ps = psum_a.tile([P, SC], F32, tag="mm512")
                    for d in range(DT):
                        nc.tensor.matmul(
                            out=ps,
                            lhsT=w_sb[d][:, t * P:(t + 1) * P],
                            rhs=x_sb[d][:, c * SC:(c + 1) * SC],
                            start=(d == 0), stop=(d == DT - 1))
                    nc.vector.tensor_scalar_add(
                        out=dst[t][:, c * SC:(c + 1) * SC],
                        in0=ps, scalar1=b_sb[:, t:t + 1])

        # V token-major, ones-augmented: [S, 4*(HD+1)]; col 64 of each
        # head group is 1.0 so the ctx matmul also produces the softmax
        # denominator Z as output row HD.
        V_sb = [persist.tile([P, HPC * (HD + 1)], BF16, tag=f"V{i}", name=f"V{i}")
                for i in range(ST)]
        for st in range(ST):
            ps = psum_a.tile([P, FPC], F32, tag="mm512")
            for d in range(DT):
                nc.tensor.matmul(
                    out=ps,
                    lhsT=vT_sb[d][:, st * P:(st + 1) * P],
                    rhs=wv_sb[d],
                    start=(d == 0), stop=(d == DT - 1))
            vv = V_sb[st].rearrange("p (h x) -> p h x", h=HPC)
            nc.vector.tensor_copy(
                out=vv[:, :, 0:HD],
                in_=ps.rearrange("p (h x) -> p h x", x=HD))
            nc.vector.memset(vv[:, :, HD:HD + 1], 1.0)

        # ---- phases B+C: attention + out-proj, interleaved per qc ---
        # ctx^T feature-major [FPC, S] bf16 (normalized)
        ctxn_sb = [persist.tile([P, S], BF16, tag=f"ctxn{t}", name=f"ctxn{t}")
                   for t in range(FT)]
        for c in range(QC):
            for hp in range(FT):      # head pair = Q^T/K^T tile index
                cps = [psum_ctx.tile([HD + 1, SC], F32, tag="ctx", name="cps")
                       for _ in range(2)]
                for kt in range(KT):
                    sc = psum_sc.tile([P, 2 * SC], F32, tag="sc")
                    ex = exp_pool.tile([P, 2 * SC], BF16, tag="ex")
                    for i in range(2):  # head 2*hp + i at partitions 64*i
                        hp0 = HD * i
                        nc.tensor.matmul(
                            out=sc[:, i * SC:(i + 1) * SC],
                            lhsT=KT_sb[hp][hp0:hp0 + HD,
                                           kt * P:(kt + 1) * P],
                            rhs=QT_sb[hp][hp0:hp0 + HD,
                                          c * SC:(c + 1) * SC],
                            start=True, stop=True,
                            tile_position=(hp0, 0))
                    nc.scalar.activation(
                        out=ex, in_=sc,
                        func=mybir.ActivationFunctionType.Exp,
                        scale=1.0 / (HD ** 0.5))
                    for i in range(2):
                        h = 2 * hp + i
                        nc.tensor.matmul(
                            out=cps[i],
                            lhsT=V_sb[kt][:, h * (HD + 1):(h + 1) * (HD + 1)],
                            rhs=ex[:, i * SC:(i + 1) * SC],
                            start=(kt == 0), stop=(kt == KT - 1))
                # Copy ctx+Z to SBUF promptly so the PSUM accumulator is
                # released for the next iteration; normalize (reciprocal,
                # DRAM-broadcast of 1/Z, multiply) runs off-critical-path.
                for i in range(2):
                    cu = z_pool.tile([HD + 1, SC], F32, tag="cu", name="cu")
                    nc.vector.tensor_copy(out=cu, in_=cps[i])
                    zr = z_pool.tile([HD + 1, SC], F32, tag="zr", name="zr")
                    nc.vector.reciprocal(
                        out=zr[HD:HD + 1, :], in_=cu[HD:HD + 1, :])
                    zd = zdram_pool.tile([1, SC], F32, tag="zd", name="zd")
                    nc.sync.dma_start(out=zd, in_=zr[HD:HD + 1, :])
                    rz = z_pool.tile([HD, SC], F32, tag="rz", name="rz")
                    nc.sync.dma_start(
                        out=rz, in_=zd.partition_broadcast(HD))
                    if i == 0:
                        nc.vector.tensor_mul(
                            out=ctxn_sb[hp][0:HD, c * SC:(c + 1) * SC],
                            in0=cu[0:HD, :], in1=rz)
                    else:
                        tmp = z_pool.tile([HD, SC], BF16, tag="tmp")
                        nc.vector.tensor_mul(
                            out=tmp, in0=cu[0:HD, :], in1=rz)
                        nc.sync.dma_start(
                            out=ctxn_sb[hp][HD:P, c * SC:(c + 1) * SC],
                            in_=tmp)
            # out-proj for this q-chunk (both head pairs now normalized)
            for fo in range(OT):
                ps = psum_a.tile([P, SC], F32, tag="mm512")
                for t in range(FT):
                    nc.tensor.matmul(
                        out=ps,
                        lhsT=wo_sb[t][:, fo * P:(fo + 1) * P],
                        rhs=ctxn_sb[t][:, c * SC:(c + 1) * SC],
                        start=(t == 0), stop=(t == FT - 1))
                ob = out_pool.tile([P, SC], F32, tag="ob")
                nc.vector.tensor_scalar_add(
                    out=ob, in0=ps, scalar1=bo_sb[:, fo:fo + 1])
                nc.sync.dma_start(
                    out=outT[fo * P:(fo + 1) * P, c * SC:(c + 1) * SC],
                    in_=ob)

    nc.compile()
    return nc


_NC_CACHE = None


def _get_program():
    global _NC_CACHE
    if _NC_CACHE is None:
        _NC_CACHE = build_program()
    return _NC_CACHE


def make_in_maps(query, key, value, Wq, bq, Wk, bk, Wv, bv, Wo, bo):
    query = np.asarray(query, np.float32)
    key = np.asarray(key, np.float32)
    value = np.asarray(value, np.float32)
    Wq, bq = np.asarray(Wq, np.float32), np.asarray(bq, np.float32)
    Wk, bk = np.asarray(Wk, np.float32), np.asarray(bk, np.float32)
    Wv, bv = np.asarray(Wv, np.float32), np.asarray(bv, np.float32)
    Wo, bo = np.asarray(Wo, np.float32), np.asarray(bo, np.float32)

    in_maps = []
    for core in range(N_CORES):
        b, hg = core // 4, core % 4
        fs = slice(FPC * hg, FPC * (hg + 1))
        in_maps.append({
            "qT": np.ascontiguousarray(query[b].T).astype(NP_BF16),
            "kT": np.ascontiguousarray(key[b].T).astype(NP_BF16),
            "vT": np.ascontiguousarray(value[b].T).astype(NP_BF16),
            "wqT": np.ascontiguousarray(Wq[fs, :].T).astype(NP_BF16),
            "wkT": np.ascontiguousarray(Wk[fs, :].T).astype(NP_BF16),
            "wvT": np.ascontiguousarray(Wv[fs, :].T).astype(NP_BF16),
            "woT": np.ascontiguousarray(Wo[:, fs].T).astype(NP_BF16),
            "bq": np.ascontiguousarray(bq[fs]),
            "bk": np.ascontiguousarray(bk[fs]),
            "bo_eff": np.ascontiguousarray(
                Wo[:, fs] @ bv[fs] + (bo if hg == 0 else 0.0)),
        })
    return in_maps


def gather_output(results):
    out = np.zeros((B, S, D), np.float32)
    for core in range(N_CORES):
        out[core // 4] += results[core]["outT"].T
    return out


def kernel(**inputs):
    from concourse.bass_utils import run_bass_kernel_spmd

    nc = _get_program()
    in_maps = make_in_maps(**inputs)
    res = run_bass_kernel_spmd(nc, in_maps, list(range(N_CORES)))
    return gather_output(res.results)


if __name__ == "__main__":
    import jax

    sys.path.insert(0, "/root/problem")
    import reference

    inputs = {k: np.asarray(v) for k, v in reference.setup_inputs().items()}
    expected = np.asarray(reference.reference(**inputs))
    actual = kernel(**inputs)
    err = np.linalg.norm(actual - expected) / np.linalg.norm(expected)
    print("Relative error:", err)


# revision 17
# speedup vs baseline: 1.0897x; 1.0036x over previous
"""MultiHeadAttention Trainium2 kernel (8 NeuronCores, SPMD).

Sharding: core = b*4 + hg where b = batch (0..1), hg = head-group (0..3).
Each core handles 4 heads (256 features) of one batch:
  Q^T/K^T = Wq/Wk column-shard proj (feature-major), V token-major,
  scores^T = K Q^T (softmax denominator via ones-augmented V matmul),
  partial out = ctx @ Wo^T row-shard.  Host sums the 4 partials per batch.

All matmuls bf16 with fp32 PSUM accumulation. exp on ScalarE (scale=1/8
fused), copies + bias adds on VectorE. Biases handled exactly:
  bq/bk: per-partition adds on Q^T/K^T (feature-major layout)
  bv, bo: folded into bo_eff = Wo[:,fsel] @ bv[fsel] + (hg==0)*bo since
          sum_k softmax = 1  =>  (ctx+bv) @ WoT = ctx @ WoT + bv @ WoT.

Emission order is tuned so ScalarE (exp is the per-core throughput
floor) starts ~30us in: project K/Q for head-pair 0, V, then attention
for pair 0 while K/Q for pair 1 projects in PE gaps.
"""

import sys

if "/opt/trn_rl_repo" not in sys.path:
    sys.path.insert(0, "/opt/trn_rl_repo")

from contextlib import ExitStack

import ml_dtypes
import numpy as np

import concourse.bass as bass
import concourse.tile as tile
from concourse import bacc, mybir

BF16 = mybir.dt.bfloat16
F32 = mybir.dt.float32
NP_BF16 = ml_dtypes.bfloat16

B, S, D = 2, 2048, 1024
H, HD = 16, 64
N_CORES = 8
HPC = 4          # heads per core
FPC = HPC * HD   # features per core = 256
P = 128
SC = 512         # q-chunk for scores/ctx matmuls (one PSUM bank)
QC = S // SC     # 4 q-chunks
DT = D // P      # 8 d-tiles (contraction tiles for projections)
KT = S // P      # 16 k-token tiles
ST = S // P      # 16 s-token tiles (V)
FT = FPC // P    # 2 feature tiles per core (Q^T/K^T, ctx)
OT = D // P      # 8 output feature tiles


def build_program():
    nc = bacc.Bacc("TRN2", target_bir_lowering=False, debug=False,
                   num_devices=N_CORES)

    qT = nc.declare_dram_parameter("qT", [D, S], BF16, isOutput=False)
    kT = nc.declare_dram_parameter("kT", [D, S], BF16, isOutput=False)
    vT = nc.declare_dram_parameter("vT", [D, S], BF16, isOutput=False)
    wqT = nc.declare_dram_parameter("wqT", [D, FPC], BF16, isOutput=False)
    wkT = nc.declare_dram_parameter("wkT", [D, FPC], BF16, isOutput=False)
    wvT = nc.declare_dram_parameter("wvT", [D, FPC], BF16, isOutput=False)
    woT = nc.declare_dram_parameter("woT", [FPC, D], BF16, isOutput=False)
    bq = nc.declare_dram_parameter("bq", [FPC], F32, isOutput=False)
    bk = nc.declare_dram_parameter("bk", [FPC], F32, isOutput=False)
    bo_eff = nc.declare_dram_parameter("bo_eff", [D], F32, isOutput=False)
    outT = nc.declare_dram_parameter("outT", [D, S], F32, isOutput=True)

    with tile.TileContext(nc) as tc, ExitStack() as ctx:
        persist = ctx.enter_context(tc.tile_pool(name="persist", bufs=1))
        psum_a = ctx.enter_context(
            tc.tile_pool(name="psum_a", bufs=2, space="PSUM"))
        psum_sc = ctx.enter_context(
            tc.tile_pool(name="psum_sc", bufs=2, space="PSUM"))
        psum_ctx = ctx.enter_context(
            tc.tile_pool(name="psum_ctx", bufs=2, space="PSUM"))
        exp_pool = ctx.enter_context(tc.tile_pool(name="exp", bufs=4))
        z_pool = ctx.enter_context(tc.tile_pool(name="z", bufs=4))
        zdram_pool = ctx.enter_context(
            tc.tile_pool(name="zdram", bufs=16, space="DRAM"))
        out_pool = ctx.enter_context(tc.tile_pool(name="out", bufs=4))

        def full_tiles(nm):
            return [persist.tile([P, S], BF16, tag=f"{nm}{i}",
                                 name=f"{nm}{i}") for i in range(DT)]

        qT_sb, kT_sb, vT_sb = full_tiles("qT"), full_tiles("kT"), \
            full_tiles("vT")
        wq_sb = [persist.tile([P, FPC], BF16, tag=f"wq{i}", name=f"wq{i}")
                 for i in range(DT)]
        wk_sb = [persist.tile([P, FPC], BF16, tag=f"wk{i}", name=f"wk{i}")
                 for i in range(DT)]
        wv_sb = [persist.tile([P, FPC], BF16, tag=f"wv{i}", name=f"wv{i}")
                 for i in range(DT)]
        wo_sb = [persist.tile([P, D], BF16, tag=f"wo{i}", name=f"wo{i}")
                 for i in range(FT)]

        # ---- loads: weights, then K/Q/V chunks in consumption order -
        for i in range(DT):
            nc.sync.dma_start(out=wk_sb[i], in_=wkT[i * P:(i + 1) * P, :])
            nc.sync.dma_start(out=wq_sb[i], in_=wqT[i * P:(i + 1) * P, :])
            nc.sync.dma_start(out=wv_sb[i], in_=wvT[i * P:(i + 1) * P, :])
        for i in range(FT):
            nc.sync.dma_start(out=wo_sb[i], in_=woT[i * P:(i + 1) * P, :])
        bq_sb = persist.tile([P, FT], F32, tag="bq")
        bk_sb = persist.tile([P, FT], F32, tag="bk")
        bo_sb = persist.tile([P, OT], F32, tag="bo")
        nc.sync.dma_start(out=bq_sb, in_=bq.rearrange("(t p) -> p t", p=P))
        nc.sync.dma_start(out=bk_sb, in_=bk.rearrange("(t p) -> p t", p=P))
        nc.sync.dma_start(out=bo_sb, in_=bo_eff.rearrange("(t p) -> p t", p=P))
        for src, dst in ((kT, kT_sb), (qT, qT_sb), (vT, vT_sb)):
            for i in range(DT):
                nc.sync.dma_start(out=dst[i],
                                  in_=src[i * P:(i + 1) * P, :])

        # ---- projection helpers -------------------------------------
        QT_sb = [persist.tile([P, S], BF16, tag=f"QT{t}", name=f"QT{t}")
                 for t in range(FT)]
        KT_sb = [persist.tile([P, S], BF16, tag=f"KT{t}", name=f"KT{t}")
                 for t in range(FT)]
        V_sb = [persist.tile([P, HPC * (HD + 1)], BF16, tag=f"V{i}",
                             name=f"V{i}") for i in range(ST)]
        ctxn_sb = [persist.tile([P, S], BF16, tag=f"ctxn{t}",
                                name=f"ctxn{t}") for t in range(FT)]

        def proj_qk(w_sb, x_sb, b_sb, dst, t):
            for c in range(QC):
                ps = psum_a.tile([P, SC], F32, tag="mm512", name="ps")
                for d in range(DT):
                    nc.tensor.matmul(
                        out=ps,
                        lhsT=w_sb[d][:, t * P:(t + 1) * P],
                        rhs=x_sb[d][:, c * SC:(c + 1) * SC],
                        start=(d == 0), stop=(d == DT - 1))
                nc.vector.tensor_scalar_add(
                    out=dst[t][:, c * SC:(c + 1) * SC],
                    in0=ps, scalar1=b_sb[:, t:t + 1])

        def proj_v(st):
            ps = psum_a.tile([P, FPC], F32, tag="mm512", name="ps")
            for d in range(DT):
                nc.tensor.matmul(
                    out=ps,
                    lhsT=vT_sb[d][:, st * P:(st + 1) * P],
                    rhs=wv_sb[d],
                    start=(d == 0), stop=(d == DT - 1))
            vv = V_sb[st].rearrange("p (h x) -> p h x", h=HPC)
            nc.vector.tensor_copy(
                out=vv[:, :, 0:HD],
                in_=ps.rearrange("p (h x) -> p h x", x=HD))
            nc.vector.memset(vv[:, :, HD:HD + 1], 1.0)

        def attention(hp, c):
            cps = [psum_ctx.tile([HD + 1, SC], F32, tag="ctx", name="cps")
                   for _ in range(2)]
            for kt in range(KT):
                sc = psum_sc.tile([P, 2 * SC], F32, tag="sc", name="sc")
                ex = exp_pool.tile([P, 2 * SC], BF16, tag="ex", name="ex")
                for i in range(2):  # head 2*hp + i at partitions 64*i
                    hp0 = HD * i
                    nc.tensor.matmul(
                        out=sc[:, i * SC:(i + 1) * SC],
                        lhsT=KT_sb[hp][hp0:hp0 + HD, kt * P:(kt + 1) * P],
                        rhs=QT_sb[hp][hp0:hp0 + HD, c * SC:(c + 1) * SC],
                        start=True, stop=True,
                        tile_position=(hp0, 0))
                nc.scalar.activation(
                    out=ex, in_=sc,
                    func=mybir.ActivationFunctionType.Exp,
                    scale=1.0 / (HD ** 0.5))
                for i in range(2):
                    h = 2 * hp + i
                    nc.tensor.matmul(
                        out=cps[i],
                        lhsT=V_sb[kt][:, h * (HD + 1):(h + 1) * (HD + 1)],
                        rhs=ex[:, i * SC:(i + 1) * SC],
                        start=(kt == 0), stop=(kt == KT - 1))
            # Copy ctx+Z to SBUF promptly so the PSUM accumulator is
            # released; normalize runs off-critical-path.
            for i in range(2):
                cu = z_pool.tile([HD + 1, SC], F32, tag="cu", name="cu")
                nc.vector.tensor_copy(out=cu, in_=cps[i])
                zr = z_pool.tile([HD + 1, SC], F32, tag="zr", name="zr")
                nc.vector.reciprocal(
                    out=zr[HD:HD + 1, :], in_=cu[HD:HD + 1, :])
                zd = zdram_pool.tile([1, SC], F32, tag="zd", name="zd")
                nc.sync.dma_start(out=zd, in_=zr[HD:HD + 1, :])
                rz = z_pool.tile([HD, SC], F32, tag="rz", name="rz")
                nc.sync.dma_start(out=rz, in_=zd.partition_broadcast(HD))
                if i == 0:
                    nc.vector.tensor_mul(
                        out=ctxn_sb[hp][0:HD, c * SC:(c + 1) * SC],
                        in0=cu[0:HD, :], in1=rz)
                else:
                    tmp = z_pool.tile([HD, SC], BF16, tag="tmp", name="tmp")
                    nc.vector.tensor_mul(
                        out=tmp, in0=cu[0:HD, :], in1=rz)
                    nc.sync.dma_start(
                        out=ctxn_sb[hp][HD:P, c * SC:(c + 1) * SC],
                        in_=tmp)

        def out_proj(c):
            for fo in range(OT):
                ps = psum_a.tile([P, SC], F32, tag="mm512", name="ps")
                for t in range(FT):
                    nc.tensor.matmul(
                        out=ps,
                        lhsT=wo_sb[t][:, fo * P:(fo + 1) * P],
                        rhs=ctxn_sb[t][:, c * SC:(c + 1) * SC],
                        start=(t == 0), stop=(t == FT - 1))
                ob = out_pool.tile([P, SC], F32, tag="ob", name="ob")
                nc.vector.tensor_scalar_add(
                    out=ob, in0=ps, scalar1=bo_sb[:, fo:fo + 1])
                nc.sync.dma_start(
                    out=outT[fo * P:(fo + 1) * P, c * SC:(c + 1) * SC],
                    in_=ob)

        # ---- emission order (= scheduling priority) -----------------
        proj_qk(wk_sb, kT_sb, bk_sb, KT_sb, 0)
        proj_qk(wq_sb, qT_sb, bq_sb, QT_sb, 0)
        for st in range(ST):
            proj_v(st)
        for c in range(QC):
            attention(0, c)
        proj_qk(wk_sb, kT_sb, bk_sb, KT_sb, 1)
        proj_qk(wq_sb, qT_sb, bq_sb, QT_sb, 1)
        for c in range(QC):
            attention(1, c)
            out_proj(c)

    nc.compile()
    return nc


_NC_CACHE = None


def _get_program():
    global _NC_CACHE
    if _NC_CACHE is None:
        _NC_CACHE = build_program()
    return _NC_CACHE


def make_in_maps(query, key, value, Wq, bq, Wk, bk, Wv, bv, Wo, bo):
    query = np.asarray(query, np.float32)
    key = np.asarray(key, np.float32)
    value = np.asarray(value, np.float32)
    Wq, bq = np.asarray(Wq, np.float32), np.asarray(bq, np.float32)
    Wk, bk = np.asarray(Wk, np.float32), np.asarray(bk, np.float32)
    Wv, bv = np.asarray(Wv, np.float32), np.asarray(bv, np.float32)
    Wo, bo = np.asarray(Wo, np.float32), np.asarray(bo, np.float32)

    in_maps = []
    for core in range(N_CORES):
        b, hg = core // 4, core % 4
        fs = slice(FPC * hg, FPC * (hg + 1))
        in_maps.append({
            "qT": np.ascontiguousarray(query[b].T).astype(NP_BF16),
            "kT": np.ascontiguousarray(key[b].T).astype(NP_BF16),
            "vT": np.ascontiguousarray(value[b].T).astype(NP_BF16),
            "wqT": np.ascontiguousarray(Wq[fs, :].T).astype(NP_BF16),
            "wkT": np.ascontiguousarray(Wk[fs, :].T).astype(NP_BF16),
            "wvT": np.ascontiguousarray(Wv[fs, :].T).astype(NP_BF16),
            "woT": np.ascontiguousarray(Wo[:, fs].T).astype(NP_BF16),
            "bq": np.ascontiguousarray(bq[fs]),
            "bk": np.ascontiguousarray(bk[fs]),
            "bo_eff": np.ascontiguousarray(
                Wo[:, fs] @ bv[fs] + (bo if hg == 0 else 0.0)),
        })
    return in_maps


def gather_output(results):
    out = np.zeros((B, S, D), np.float32)
    for core in range(N_CORES):
        out[core // 4] += results[core]["outT"].T
    return out


def kernel(**inputs):
    from concourse.bass_utils import run_bass_kernel_spmd

    nc = _get_program()
    in_maps = make_in_maps(**inputs)
    res = run_bass_kernel_spmd(nc, in_maps, list(range(N_CORES)))
    return gather_output(res.results)


if __name__ == "__main__":
    import jax

    sys.path.insert(0, "/root/problem")
    import reference

    inputs = {k: np.asarray(v) for k, v in reference.setup_inputs().items()}
    expected = np.asarray(reference.reference(**inputs))
    actual = kernel(**inputs)
    err = np.linalg.norm(actual - expected) / np.linalg.norm(expected)
    print("Relative error:", err)


# revision 18
# speedup vs baseline: 1.2645x; 1.1604x over previous
"""MultiHeadAttention Trainium2 kernel (8 NeuronCores, SPMD).

Sharding: core = b*4 + hg where b = batch (0..1), hg = head-group (0..3).
Each core handles 4 heads (256 features) of one batch:
  Q^T/K^T = Wq/Wk column-shard proj (feature-major), V token-major,
  scores^T = K Q^T (softmax denominator via ones-augmented V matmul),
  partial out = ctx @ Wo^T row-shard.  Host sums the 4 partials per batch.

All matmuls bf16 with fp32 PSUM accumulation. exp on ScalarE (scale=1/8
fused), copies + bias adds on VectorE. Biases handled exactly:
  bq/bk: per-partition adds on Q^T/K^T (feature-major layout)
  bv, bo: folded into bo_eff = Wo[:,fsel] @ bv[fsel] + (hg==0)*bo since
          sum_k softmax = 1  =>  (ctx+bv) @ WoT = ctx @ WoT + bv @ WoT.

Emission order is tuned so ScalarE (exp is the per-core throughput
floor) starts ~30us in: project K/Q for head-pair 0, V, then attention
for pair 0 while K/Q for pair 1 projects in PE gaps.
"""

import sys

if "/opt/trn_rl_repo" not in sys.path:
    sys.path.insert(0, "/opt/trn_rl_repo")

from contextlib import ExitStack

import ml_dtypes
import numpy as np

import concourse.bass as bass
import concourse.tile as tile
from concourse import bacc, mybir

BF16 = mybir.dt.bfloat16
F32 = mybir.dt.float32
NP_BF16 = ml_dtypes.bfloat16

B, S, D = 2, 2048, 1024
H, HD = 16, 64
N_CORES = 8
HPC = 4          # heads per core
FPC = HPC * HD   # features per core = 256
P = 128
SC = 512         # q-chunk for scores/ctx matmuls (one PSUM bank)
QC = S // SC     # 4 q-chunks
DT = D // P      # 8 d-tiles (contraction tiles for projections)
KT = S // P      # 16 k-token tiles
ST = S // P      # 16 s-token tiles (V)
FT = FPC // P    # 2 feature tiles per core (Q^T/K^T, ctx)
OT = D // P      # 8 output feature tiles


def build_program():
    nc = bacc.Bacc("TRN2", target_bir_lowering=False, debug=False,
                   num_devices=N_CORES)

    qT = nc.declare_dram_parameter("qT", [D, S], BF16, isOutput=False)
    kT = nc.declare_dram_parameter("kT", [D, S], BF16, isOutput=False)
    vT = nc.declare_dram_parameter("vT", [D, S], BF16, isOutput=False)
    wqT = nc.declare_dram_parameter("wqT", [D, FPC], BF16, isOutput=False)
    wkT = nc.declare_dram_parameter("wkT", [D, FPC], BF16, isOutput=False)
    wvT = nc.declare_dram_parameter("wvT", [D, FPC], BF16, isOutput=False)
    woT = nc.declare_dram_parameter("woT", [FPC, D], BF16, isOutput=False)
    bq = nc.declare_dram_parameter("bq", [FPC], F32, isOutput=False)
    bk = nc.declare_dram_parameter("bk", [FPC], F32, isOutput=False)
    bo_eff = nc.declare_dram_parameter("bo_eff", [D], F32, isOutput=False)
    outT = nc.declare_dram_parameter("outT", [D, S], F32, isOutput=True)

    with tile.TileContext(nc) as tc, ExitStack() as ctx:
        persist = ctx.enter_context(tc.tile_pool(name="persist", bufs=1))
        psum_a = ctx.enter_context(
            tc.tile_pool(name="psum_a", bufs=2, space="PSUM"))
        psum_sc = ctx.enter_context(
            tc.tile_pool(name="psum_sc", bufs=2, space="PSUM"))
        psum_ctx = ctx.enter_context(
            tc.tile_pool(name="psum_ctx", bufs=2, space="PSUM"))
        exp_pool = ctx.enter_context(tc.tile_pool(name="exp", bufs=4))
        z_pool = ctx.enter_context(tc.tile_pool(name="z", bufs=4))
        zdram_pool = ctx.enter_context(
            tc.tile_pool(name="zdram", bufs=16, space="DRAM"))
        out_pool = ctx.enter_context(tc.tile_pool(name="out", bufs=4))

        def full_tiles(nm):
            return [persist.tile([P, S], BF16, tag=f"{nm}{i}",
                                 name=f"{nm}{i}") for i in range(DT)]

        qT_sb, kT_sb, vT_sb = full_tiles("qT"), full_tiles("kT"), \
            full_tiles("vT")
        wq_sb = [persist.tile([P, FPC], BF16, tag=f"wq{i}", name=f"wq{i}")
                 for i in range(DT)]
        wk_sb = [persist.tile([P, FPC], BF16, tag=f"wk{i}", name=f"wk{i}")
                 for i in range(DT)]
        wv_sb = [persist.tile([P, FPC], BF16, tag=f"wv{i}", name=f"wv{i}")
                 for i in range(DT)]
        wo_sb = [persist.tile([P, D], BF16, tag=f"wo{i}", name=f"wo{i}")
                 for i in range(FT)]

        # ---- loads: weights, then K/Q/V chunks in consumption order -
        for i in range(DT):
            nc.sync.dma_start(out=wk_sb[i], in_=wkT[i * P:(i + 1) * P, :])
            nc.sync.dma_start(out=wq_sb[i], in_=wqT[i * P:(i + 1) * P, :])
            nc.sync.dma_start(out=wv_sb[i], in_=wvT[i * P:(i + 1) * P, :])
        for i in range(FT):
            nc.sync.dma_start(out=wo_sb[i], in_=woT[i * P:(i + 1) * P, :])
        bq_sb = persist.tile([P, FT], F32, tag="bq")
        bk_sb = persist.tile([P, FT], F32, tag="bk")
        bo_sb = persist.tile([P, OT], F32, tag="bo")
        nc.sync.dma_start(out=bq_sb, in_=bq.rearrange("(t p) -> p t", p=P))
        nc.sync.dma_start(out=bk_sb, in_=bk.rearrange("(t p) -> p t", p=P))
        nc.sync.dma_start(out=bo_sb, in_=bo_eff.rearrange("(t p) -> p t", p=P))
        for src, dst in ((kT, kT_sb), (qT, qT_sb), (vT, vT_sb)):
            for i in range(DT):
                nc.sync.dma_start(out=dst[i],
                                  in_=src[i * P:(i + 1) * P, :])

        # ---- projection helpers -------------------------------------
        QT_sb = [persist.tile([P, S], BF16, tag=f"QT{t}", name=f"QT{t}")
                 for t in range(FT)]
        KT_sb = [persist.tile([P, S], BF16, tag=f"KT{t}", name=f"KT{t}")
                 for t in range(FT)]
        V_sb = [persist.tile([P, HPC * (HD + 1)], BF16, tag=f"V{i}",
                             name=f"V{i}") for i in range(ST)]
        ctxn_sb = [persist.tile([P, S], BF16, tag=f"ctxn{t}",
                                name=f"ctxn{t}") for t in range(FT)]

        def proj_qk(w_sb, x_sb, b_sb, dst, t):
            for c in range(QC):
                ps = psum_a.tile([P, SC], F32, tag="mm512", name="ps")
                for d in range(DT):
                    nc.tensor.matmul(
                        out=ps,
                        lhsT=w_sb[d][:, t * P:(t + 1) * P],
                        rhs=x_sb[d][:, c * SC:(c + 1) * SC],
                        start=(d == 0), stop=(d == DT - 1))
                nc.vector.tensor_scalar_add(
                    out=dst[t][:, c * SC:(c + 1) * SC],
                    in0=ps, scalar1=b_sb[:, t:t + 1])

        def proj_v(st):
            ps = psum_a.tile([P, FPC], F32, tag="mm512", name="ps")
            for d in range(DT):
                nc.tensor.matmul(
                    out=ps,
                    lhsT=vT_sb[d][:, st * P:(st + 1) * P],
                    rhs=wv_sb[d],
                    start=(d == 0), stop=(d == DT - 1))
            vv = V_sb[st].rearrange("p (h x) -> p h x", h=HPC)
            nc.vector.tensor_copy(
                out=vv[:, :, 0:HD],
                in_=ps.rearrange("p (h x) -> p h x", x=HD))
            nc.vector.memset(vv[:, :, HD:HD + 1], 1.0)

        def attention(hp, c):
            cps = [psum_ctx.tile([HD + 1, SC], F32, tag="ctx", name="cps")
                   for _ in range(2)]
            for kt in range(KT):
                sc = psum_sc.tile([P, 2 * SC], F32, tag="sc", name="sc")
                ex = exp_pool.tile([P, 2 * SC], BF16, tag="ex", name="ex")
                for i in range(2):  # head 2*hp + i at partitions 64*i
                    hp0 = HD * i
                    nc.tensor.matmul(
                        out=sc[:, i * SC:(i + 1) * SC],
                        lhsT=KT_sb[hp][hp0:hp0 + HD, kt * P:(kt + 1) * P],
                        rhs=QT_sb[hp][hp0:hp0 + HD, c * SC:(c + 1) * SC],
                        start=True, stop=True)
                nc.scalar.activation(
                    out=ex, in_=sc,
                    func=mybir.ActivationFunctionType.Exp,
                    scale=1.0 / (HD ** 0.5))
                for i in range(2):
                    h = 2 * hp + i
                    nc.tensor.matmul(
                        out=cps[i],
                        lhsT=V_sb[kt][:, h * (HD + 1):(h + 1) * (HD + 1)],
                        rhs=ex[:, i * SC:(i + 1) * SC],
                        start=(kt == 0), stop=(kt == KT - 1))
            # Copy ctx+Z to SBUF promptly so the PSUM accumulator is
            # released; normalize runs off-critical-path.
            for i in range(2):
                cu = z_pool.tile([HD + 1, SC], F32, tag="cu", name="cu")
                nc.vector.tensor_copy(out=cu, in_=cps[i])
                zr = z_pool.tile([HD + 1, SC], F32, tag="zr", name="zr")
                nc.vector.reciprocal(
                    out=zr[HD:HD + 1, :], in_=cu[HD:HD + 1, :])
                zd = zdram_pool.tile([1, SC], F32, tag="zd", name="zd")
                nc.sync.dma_start(out=zd, in_=zr[HD:HD + 1, :])
                rz = z_pool.tile([HD, SC], F32, tag="rz", name="rz")
                nc.sync.dma_start(out=rz, in_=zd.partition_broadcast(HD))
                if i == 0:
                    nc.vector.tensor_mul(
                        out=ctxn_sb[hp][0:HD, c * SC:(c + 1) * SC],
                        in0=cu[0:HD, :], in1=rz)
                else:
                    tmp = z_pool.tile([HD, SC], BF16, tag="tmp", name="tmp")
                    nc.vector.tensor_mul(
                        out=tmp, in0=cu[0:HD, :], in1=rz)
                    nc.sync.dma_start(
                        out=ctxn_sb[hp][HD:P, c * SC:(c + 1) * SC],
                        in_=tmp)

        def out_proj(c):
            for fo in range(OT):
                ps = psum_a.tile([P, SC], F32, tag="mm512", name="ps")
                for t in range(FT):
                    nc.tensor.matmul(
                        out=ps,
                        lhsT=wo_sb[t][:, fo * P:(fo + 1) * P],
                        rhs=ctxn_sb[t][:, c * SC:(c + 1) * SC],
                        start=(t == 0), stop=(t == FT - 1))
                ob = out_pool.tile([P, SC], F32, tag="ob", name="ob")
                nc.vector.tensor_scalar_add(
                    out=ob, in0=ps, scalar1=bo_sb[:, fo:fo + 1])
                nc.sync.dma_start(
                    out=outT[fo * P:(fo + 1) * P, c * SC:(c + 1) * SC],
                    in_=ob)

        # ---- emission order (= scheduling priority) -----------------
        proj_qk(wk_sb, kT_sb, bk_sb, KT_sb, 0)
        proj_qk(wq_sb, qT_sb, bq_sb, QT_sb, 0)
        for st in range(ST):
            proj_v(st)
        for c in range(QC):
            attention(0, c)
        proj_qk(wk_sb, kT_sb, bk_sb, KT_sb, 1)
        proj_qk(wq_sb, qT_sb, bq_sb, QT_sb, 1)
        for c in range(QC):
            attention(1, c)
            out_proj(c)

    nc.compile()
    return nc


_NC_CACHE = None


def _get_program():
    global _NC_CACHE
    if _NC_CACHE is None:
        _NC_CACHE = build_program()
    return _NC_CACHE


def make_in_maps(query, key, value, Wq, bq, Wk, bk, Wv, bv, Wo, bo):
    query = np.asarray(query, np.float32)
    key = np.asarray(key, np.float32)
    value = np.asarray(value, np.float32)
    Wq, bq = np.asarray(Wq, np.float32), np.asarray(bq, np.float32)
    Wk, bk = np.asarray(Wk, np.float32), np.asarray(bk, np.float32)
    Wv, bv = np.asarray(Wv, np.float32), np.asarray(bv, np.float32)
    Wo, bo = np.asarray(Wo, np.float32), np.asarray(bo, np.float32)

    in_maps = []
    for core in range(N_CORES):
        b, hg = core // 4, core % 4
        fs = slice(FPC * hg, FPC * (hg + 1))
        in_maps.append({
            "qT": np.ascontiguousarray(query[b].T).astype(NP_BF16),
            "kT": np.ascontiguousarray(key[b].T).astype(NP_BF16),
            "vT": np.ascontiguousarray(value[b].T).astype(NP_BF16),
            "wqT": np.ascontiguousarray(Wq[fs, :].T).astype(NP_BF16),
            "wkT": np.ascontiguousarray(Wk[fs, :].T).astype(NP_BF16),
            "wvT": np.ascontiguousarray(Wv[fs, :].T).astype(NP_BF16),
            "woT": np.ascontiguousarray(Wo[:, fs].T).astype(NP_BF16),
            "bq": np.ascontiguousarray(bq[fs]),
            "bk": np.ascontiguousarray(bk[fs]),
            "bo_eff": np.ascontiguousarray(
                Wo[:, fs] @ bv[fs] + (bo if hg == 0 else 0.0)),
        })
    return in_maps


def gather_output(results):
    out = np.zeros((B, S, D), np.float32)
    for core in range(N_CORES):
        out[core // 4] += results[core]["outT"].T
    return out


def kernel(**inputs):
    from concourse.bass_utils import run_bass_kernel_spmd

    nc = _get_program()
    in_maps = make_in_maps(**inputs)
    res = run_bass_kernel_spmd(nc, in_maps, list(range(N_CORES)))
    return gather_output(res.results)


if __name__ == "__main__":
    import jax

    sys.path.insert(0, "/root/problem")
    import reference

    inputs = {k: np.asarray(v) for k, v in reference.setup_inputs().items()}
    expected = np.asarray(reference.reference(**inputs))
    actual = kernel(**inputs)
    err = np.linalg.norm(actual - expected) / np.linalg.norm(expected)
    print("Relative error:", err)


# revision 19
# speedup vs baseline: 1.2669x; 1.0019x over previous
"""MultiHeadAttention Trainium2 kernel (8 NeuronCores, SPMD).

Sharding: core = b*4 + hg where b = batch (0..1), hg = head-group (0..3).
Each core handles 4 heads (256 features) of one batch:
  Q^T/K^T = Wq/Wk column-shard proj (feature-major), V token-major,
  scores^T = K Q^T (softmax denominator via ones-augmented V matmul),
  partial out = ctx @ Wo^T row-shard.  Host sums the 4 partials per batch.

All matmuls bf16 with fp32 PSUM accumulation. exp on ScalarE (scale=1/8
fused), copies + bias adds on VectorE. Biases handled exactly:
  bq/bk: per-partition adds on Q^T/K^T (feature-major layout)
  bv, bo: folded into bo_eff = Wo[:,fsel] @ bv[fsel] + (hg==0)*bo since
          sum_k softmax = 1  =>  (ctx+bv) @ WoT = ctx @ WoT + bv @ WoT.

Emission order is tuned so ScalarE (exp is the per-core throughput
floor) starts ~30us in: project K/Q for head-pair 0, V, then attention
for pair 0 while K/Q for pair 1 projects in PE gaps.
"""

import sys

if "/opt/trn_rl_repo" not in sys.path:
    sys.path.insert(0, "/opt/trn_rl_repo")

from contextlib import ExitStack

import ml_dtypes
import numpy as np

import concourse.bass as bass
import concourse.tile as tile
from concourse import bacc, mybir

BF16 = mybir.dt.bfloat16
F32 = mybir.dt.float32
NP_BF16 = ml_dtypes.bfloat16

B, S, D = 2, 2048, 1024
H, HD = 16, 64
N_CORES = 8
HPC = 4          # heads per core
FPC = HPC * HD   # features per core = 256
P = 128
SC = 512         # q-chunk for scores/ctx matmuls (one PSUM bank)
QC = S // SC     # 4 q-chunks
DT = D // P      # 8 d-tiles (contraction tiles for projections)
KT = S // P      # 16 k-token tiles
ST = S // P      # 16 s-token tiles (V)
FT = FPC // P    # 2 feature tiles per core (Q^T/K^T, ctx)
OT = D // P      # 8 output feature tiles


def build_program():
    nc = bacc.Bacc("TRN2", target_bir_lowering=False, debug=False,
                   num_devices=N_CORES)

    qT = nc.declare_dram_parameter("qT", [D, S], BF16, isOutput=False)
    kT = nc.declare_dram_parameter("kT", [D, S], BF16, isOutput=False)
    vT = nc.declare_dram_parameter("vT", [D, S], BF16, isOutput=False)
    wqT = nc.declare_dram_parameter("wqT", [D, FPC], BF16, isOutput=False)
    wkT = nc.declare_dram_parameter("wkT", [D, FPC], BF16, isOutput=False)
    wvT = nc.declare_dram_parameter("wvT", [D, FPC], BF16, isOutput=False)
    woT = nc.declare_dram_parameter("woT", [FPC, D], BF16, isOutput=False)
    bq = nc.declare_dram_parameter("bq", [FPC], F32, isOutput=False)
    bk = nc.declare_dram_parameter("bk", [FPC], F32, isOutput=False)
    bo_eff = nc.declare_dram_parameter("bo_eff", [D], F32, isOutput=False)
    outT = nc.declare_dram_parameter("outT", [D, S], F32, isOutput=True)

    with tile.TileContext(nc) as tc, ExitStack() as ctx:
        persist = ctx.enter_context(tc.tile_pool(name="persist", bufs=1))
        psum_a = ctx.enter_context(
            tc.tile_pool(name="psum_a", bufs=2, space="PSUM"))
        psum_sc = ctx.enter_context(
            tc.tile_pool(name="psum_sc", bufs=2, space="PSUM"))
        psum_ctx = ctx.enter_context(
            tc.tile_pool(name="psum_ctx", bufs=2, space="PSUM"))
        exp_pool = ctx.enter_context(tc.tile_pool(name="exp", bufs=4))
        z_pool = ctx.enter_context(tc.tile_pool(name="z", bufs=4))
        zdram_pool = ctx.enter_context(
            tc.tile_pool(name="zdram", bufs=16, space="DRAM"))
        out_pool = ctx.enter_context(tc.tile_pool(name="out", bufs=4))

        def full_tiles(nm):
            return [persist.tile([P, S], BF16, tag=f"{nm}{i}",
                                 name=f"{nm}{i}") for i in range(DT)]

        qT_sb, kT_sb, vT_sb = full_tiles("qT"), full_tiles("kT"), \
            full_tiles("vT")
        wq_sb = [persist.tile([P, FPC], BF16, tag=f"wq{i}", name=f"wq{i}")
                 for i in range(DT)]
        wk_sb = [persist.tile([P, FPC], BF16, tag=f"wk{i}", name=f"wk{i}")
                 for i in range(DT)]
        wv_sb = [persist.tile([P, FPC], BF16, tag=f"wv{i}", name=f"wv{i}")
                 for i in range(DT)]
        wo_sb = [persist.tile([P, D], BF16, tag=f"wo{i}", name=f"wo{i}")
                 for i in range(FT)]

        # ---- loads: weights, then K/Q/V chunks in consumption order -
        for i in range(DT):
            nc.sync.dma_start(out=wk_sb[i], in_=wkT[i * P:(i + 1) * P, :])
            nc.sync.dma_start(out=wq_sb[i], in_=wqT[i * P:(i + 1) * P, :])
            nc.sync.dma_start(out=wv_sb[i], in_=wvT[i * P:(i + 1) * P, :])
        for i in range(FT):
            nc.sync.dma_start(out=wo_sb[i], in_=woT[i * P:(i + 1) * P, :])
        bq_sb = persist.tile([P, FT], F32, tag="bq")
        bk_sb = persist.tile([P, FT], F32, tag="bk")
        bo_sb = persist.tile([P, OT], F32, tag="bo")
        nc.sync.dma_start(out=bq_sb, in_=bq.rearrange("(t p) -> p t", p=P))
        nc.sync.dma_start(out=bk_sb, in_=bk.rearrange("(t p) -> p t", p=P))
        nc.sync.dma_start(out=bo_sb, in_=bo_eff.rearrange("(t p) -> p t", p=P))
        for src, dst in ((kT, kT_sb), (qT, qT_sb), (vT, vT_sb)):
            for i in range(DT):
                nc.sync.dma_start(out=dst[i],
                                  in_=src[i * P:(i + 1) * P, :])

        # ---- projection helpers -------------------------------------
        QT_sb = [persist.tile([P, S], BF16, tag=f"QT{t}", name=f"QT{t}")
                 for t in range(FT)]
        KT_sb = [persist.tile([P, S], BF16, tag=f"KT{t}", name=f"KT{t}")
                 for t in range(FT)]
        V_sb = [persist.tile([P, HPC * (HD + 1)], BF16, tag=f"V{i}",
                             name=f"V{i}") for i in range(ST)]
        ctxn_sb = [persist.tile([P, S], BF16, tag=f"ctxn{t}",
                                name=f"ctxn{t}") for t in range(FT)]

        def proj_qk(w_sb, x_sb, b_sb, dst, t):
            for c in range(QC):
                ps = psum_a.tile([P, SC], F32, tag="mm512", name="ps")
                for d in range(DT):
                    nc.tensor.matmul(
                        out=ps,
                        lhsT=w_sb[d][:, t * P:(t + 1) * P],
                        rhs=x_sb[d][:, c * SC:(c + 1) * SC],
                        start=(d == 0), stop=(d == DT - 1))
                nc.vector.tensor_scalar_add(
                    out=dst[t][:, c * SC:(c + 1) * SC],
                    in0=ps, scalar1=b_sb[:, t:t + 1])

        def proj_v(st):
            ps = psum_a.tile([P, FPC], F32, tag="mm512", name="ps")
            for d in range(DT):
                nc.tensor.matmul(
                    out=ps,
                    lhsT=vT_sb[d][:, st * P:(st + 1) * P],
                    rhs=wv_sb[d],
                    start=(d == 0), stop=(d == DT - 1))
            vv = V_sb[st].rearrange("p (h x) -> p h x", h=HPC)
            nc.vector.tensor_copy(
                out=vv[:, :, 0:HD],
                in_=ps.rearrange("p (h x) -> p h x", x=HD))
            nc.vector.memset(vv[:, :, HD:HD + 1], 1.0)

        def attention(hp, c):
            cps = [psum_ctx.tile([HD + 1, SC], F32, tag="ctx", name="cps")
                   for _ in range(2)]
            for kt in range(KT):
                sc = psum_sc.tile([P, 2 * SC], F32, tag="sc", name="sc")
                ex = exp_pool.tile([P, 2 * SC], BF16, tag="ex", name="ex")
                for i in range(2):  # head 2*hp + i at partitions 64*i
                    hp0 = HD * i
                    nc.tensor.matmul(
                        out=sc[:, i * SC:(i + 1) * SC],
                        lhsT=KT_sb[hp][hp0:hp0 + HD, kt * P:(kt + 1) * P],
                        rhs=QT_sb[hp][hp0:hp0 + HD, c * SC:(c + 1) * SC],
                        start=True, stop=True)
                nc.scalar.activation(
                    out=ex, in_=sc,
                    func=mybir.ActivationFunctionType.Exp,
                    scale=1.0 / (HD ** 0.5))
                for i in range(2):
                    h = 2 * hp + i
                    nc.tensor.matmul(
                        out=cps[i],
                        lhsT=V_sb[kt][:, h * (HD + 1):(h + 1) * (HD + 1)],
                        rhs=ex[:, i * SC:(i + 1) * SC],
                        start=(kt == 0), stop=(kt == KT - 1))
            # Copy ctx+Z to SBUF promptly so the PSUM accumulator is
            # released; normalize runs off-critical-path.
            for i in range(2):
                cu = z_pool.tile([HD + 1, SC], F32, tag="cu", name="cu")
                nc.vector.tensor_copy(out=cu, in_=cps[i])
                zd = zdram_pool.tile([1, SC], F32, tag="zd", name="zd")
                nc.sync.dma_start(out=zd, in_=cu[HD:HD + 1, :])
                zb = z_pool.tile([HD, SC], F32, tag="zb", name="zb")
                nc.sync.dma_start(out=zb, in_=zd.partition_broadcast(HD))
                rz = z_pool.tile([HD, SC], F32, tag="rz", name="rz")
                nc.vector.reciprocal(out=rz, in_=zb)
                if i == 0:
                    nc.vector.tensor_mul(
                        out=ctxn_sb[hp][0:HD, c * SC:(c + 1) * SC],
                        in0=cu[0:HD, :], in1=rz)
                else:
                    tmp = z_pool.tile([HD, SC], BF16, tag="tmp", name="tmp")
                    nc.vector.tensor_mul(
                        out=tmp, in0=cu[0:HD, :], in1=rz)
                    nc.sync.dma_start(
                        out=ctxn_sb[hp][HD:P, c * SC:(c + 1) * SC],
                        in_=tmp)

        def out_proj(c):
            for fo in range(OT):
                ps = psum_a.tile([P, SC], F32, tag="mm512", name="ps")
                for t in range(FT):
                    nc.tensor.matmul(
                        out=ps,
                        lhsT=wo_sb[t][:, fo * P:(fo + 1) * P],
                        rhs=ctxn_sb[t][:, c * SC:(c + 1) * SC],
                        start=(t == 0), stop=(t == FT - 1))
                ob = out_pool.tile([P, SC], F32, tag="ob", name="ob")
                nc.vector.tensor_scalar_add(
                    out=ob, in0=ps, scalar1=bo_sb[:, fo:fo + 1])
                nc.sync.dma_start(
                    out=outT[fo * P:(fo + 1) * P, c * SC:(c + 1) * SC],
                    in_=ob)

        # ---- emission order (= scheduling priority) -----------------
        proj_qk(wk_sb, kT_sb, bk_sb, KT_sb, 0)
        proj_qk(wq_sb, qT_sb, bq_sb, QT_sb, 0)
        for st in range(ST):
            proj_v(st)
        for c in range(QC):
            attention(0, c)
        proj_qk(wk_sb, kT_sb, bk_sb, KT_sb, 1)
        proj_qk(wq_sb, qT_sb, bq_sb, QT_sb, 1)
        for c in range(QC):
            attention(1, c)
            out_proj(c)

    nc.compile()
    return nc


_NC_CACHE = None


def _get_program():
    global _NC_CACHE
    if _NC_CACHE is None:
        _NC_CACHE = build_program()
    return _NC_CACHE


def make_in_maps(query, key, value, Wq, bq, Wk, bk, Wv, bv, Wo, bo):
    query = np.asarray(query, np.float32)
    key = np.asarray(key, np.float32)
    value = np.asarray(value, np.float32)
    Wq, bq = np.asarray(Wq, np.float32), np.asarray(bq, np.float32)
    Wk, bk = np.asarray(Wk, np.float32), np.asarray(bk, np.float32)
    Wv, bv = np.asarray(Wv, np.float32), np.asarray(bv, np.float32)
    Wo, bo = np.asarray(Wo, np.float32), np.asarray(bo, np.float32)

    in_maps = []
    for core in range(N_CORES):
        b, hg = core // 4, core % 4
        fs = slice(FPC * hg, FPC * (hg + 1))
        in_maps.append({
            "qT": np.ascontiguousarray(query[b].T).astype(NP_BF16),
            "kT": np.ascontiguousarray(key[b].T).astype(NP_BF16),
            "vT": np.ascontiguousarray(value[b].T).astype(NP_BF16),
            "wqT": np.ascontiguousarray(Wq[fs, :].T).astype(NP_BF16),
            "wkT": np.ascontiguousarray(Wk[fs, :].T).astype(NP_BF16),
            "wvT": np.ascontiguousarray(Wv[fs, :].T).astype(NP_BF16),
            "woT": np.ascontiguousarray(Wo[:, fs].T).astype(NP_BF16),
            "bq": np.ascontiguousarray(bq[fs]),
            "bk": np.ascontiguousarray(bk[fs]),
            "bo_eff": np.ascontiguousarray(
                Wo[:, fs] @ bv[fs] + (bo if hg == 0 else 0.0)),
        })
    return in_maps


def gather_output(results):
    out = np.zeros((B, S, D), np.float32)
    for core in range(N_CORES):
        out[core // 4] += results[core]["outT"].T
    return out


def kernel(**inputs):
    from concourse.bass_utils import run_bass_kernel_spmd

    nc = _get_program()
    in_maps = make_in_maps(**inputs)
    res = run_bass_kernel_spmd(nc, in_maps, list(range(N_CORES)))
    return gather_output(res.results)


if __name__ == "__main__":
    import jax

    sys.path.insert(0, "/root/problem")
    import reference

    inputs = {k: np.asarray(v) for k, v in reference.setup_inputs().items()}
    expected = np.asarray(reference.reference(**inputs))
    actual = kernel(**inputs)
    err = np.linalg.norm(actual - expected) / np.linalg.norm(expected)
    print("Relative error:", err)
